# revision 1
# baseline (speedup 1.0000x reference)
"""Trainium2 Bass kernel for nn_AttConvModule (depthwise conv3d + BN + ReLU +
adaptive maxpool + grouped 1x1 attention), data-parallel over batch B=8 on 8
NeuronCores.

Per-core pipeline (batch element b on core b):
  1. Host pre-casts x to bf16 (halves HBM traffic; SWDGE cast-DMA measured
     ~3x slower than HWDGE); stream x[b] via HWDGE, double-buffered d-quads.
  2. Depthwise 3x3 conv as diagonal-matmul taps on the PE (BN scale folded
     into the tap weights), f32 PSUM accumulation per 8-row chunk. Two taps
     ((0,0) and (2,0)) are folded into the (1,0) "carrier" tap: ScalarE+DVE
     build P = x_58 + r0*x_0 + r6*x_116 (r = weight ratios, exact algebra),
     so the PE runs 7 streaming passes instead of 9.
  3. Fused W/H/D maxpool (8x8 spatial x 4 depth) as one DVE tensor_reduce
     per 4-bank chunk group, straight out of PSUM.
  4. Bias+ReLU on the pooled (tiny) tensor via ScalarE (bias folds conv
     bias + BN shift; pooling commutes with the monotone bias+relu).
  5. Attention tail on PE/DVE/ScalarE in f32: grouped 1x1 convs g/f/h, the
     row-major (C2,N)->(N,C2) reshape via a DRAM round trip, scores matmul,
     softmax, output matmul.
"""
import numpy as np
import ml_dtypes

import concourse.bass as bass
import concourse.tile as tile
from concourse import bacc, mybir
from concourse.bass_utils import run_bass_kernel_spmd

F32 = mybir.dt.float32
BF16 = mybir.dt.bfloat16
AX = mybir.AxisListType
AF = mybir.ActivationFunctionType

# Problem geometry (hardcoded per contract)
B, C, D, H, W = 8, 512, 16, 58, 58
C2 = C // 2
Do, Ho, Wo = 4, 7, 7
N = Do * Ho * Wo          # 196
HW = H * W                # 3364
CB = 4                    # channel blocks of 128
DQ = 4                    # d-quads (== d-groups of the pool)
HB = 7                    # 8-row output chunks per d-slice
EPS = 1e-5

_CACHE = {}


def _build_nc():
    nc = bacc.Bacc("TRN2", target_bir_lowering=False, debug=False, num_devices=8)

    x_d = nc.dram_tensor("x", [C, D, HW], BF16, kind="ExternalInput").ap()
    dg_d = nc.dram_tensor("dg", [128, CB * 9 * 128], BF16, kind="ExternalInput").ap()
    bias_d = nc.dram_tensor("bias", [128, CB], F32, kind="ExternalInput").ap()
    attw_d = nc.dram_tensor("attw", [128, 12 * 64], F32, kind="ExternalInput").ap()
    ident_d = nc.dram_tensor("ident", [128, 128], F32, kind="ExternalInput").ap()
    rvec_d = nc.dram_tensor("rvec", [128, 12], F32, kind="ExternalInput").ap()
    gflat_d = nc.dram_tensor("gflat", [C2 * N], F32).ap()
    out_d = nc.dram_tensor("out", [C2, N], F32, kind="ExternalOutput").ap()
    LP = 56 * W  # carrier window length (3248)

    with tile.TileContext(nc) as tc:
        with (
            tc.tile_pool(name="consts", bufs=1) as consts,
            tc.tile_pool(name="ys", bufs=1) as ysp,
        ):
            dg_sb = consts.tile([128, CB * 9 * 128], BF16)
            nc.sync.dma_start(dg_sb[:], dg_d[:])
            bias_sb = consts.tile([128, CB], F32)
            nc.sync.dma_start(bias_sb[:], bias_d[:])
            attw_sb = consts.tile([128, 12 * 64], F32)
            nc.sync.dma_start(attw_sb[:], attw_d[:])
            ident_sb = consts.tile([128, 128], F32)
            nc.sync.dma_start(ident_sb[:], ident_d[:])
            rvec_sb = consts.tile([128, 12], F32)
            nc.sync.dma_start(rvec_sb[:], rvec_d[:])
            # warm the ACT exp table during conv (off the critical tail path)
            warm = consts.tile([128, 1], F32)
            nc.scalar.activation(warm[:], rvec_sb[:, 0:1], AF.Exp,
                                 bias=0.0, scale=0.0)

            # pooled conv output, one tile per channel block: [128, 4*49]
            y_t = [ysp.tile([128, N], F32, tag=f"y{cb}", name=f"y{cb}") for cb in range(CB)]
            # post bias+relu
            y2_t = [ysp.tile([128, N], F32, tag=f"y2{cb}", name=f"y2{cb}") for cb in range(CB)]

            # ---------------- conv + pool ----------------
            # taps 0=(0,0) and 6=(2,0) fold into carrier tap 3=(1,0)
            PE_TAPS = [1, 2, 3, 4, 5, 7, 8]
            with (
                tc.tile_pool(name="xq", bufs=2) as xq,
                tc.tile_pool(name="pp", bufs=8) as pp,
                tc.tile_pool(name="tp", bufs=8) as tp,
                tc.tile_pool(name="cps", bufs=2, space="PSUM") as cps,
            ):
                # software pipeline: load x + build carrier P one quad ahead of
                # the PE/reduce stage, so DVE's P-adds precede the previous
                # quad's reduces in DVE program order.
                NIT = CB * DQ

                def build_P_dd(it, xt, dd):
                    cb = it // DQ
                    t0 = tp.tile([128, LP], BF16, tag="t", name="t0")
                    nc.scalar.mul(t0[:], xt[:, dd, 0:LP],
                                  rvec_sb[:, 2 * cb:2 * cb + 1])
                    t1 = tp.tile([128, LP], BF16, tag="t", name="t1")
                    nc.scalar.mul(t1[:], xt[:, dd, 116:116 + LP],
                                  rvec_sb[:, 2 * cb + 1:2 * cb + 2])
                    P = pp.tile([128, LP], BF16, tag="P", name=f"P{dd}")
                    nc.vector.tensor_add(P[:], t0[:], t1[:])
                    nc.vector.tensor_add(P[:], P[:], xt[:, dd, 58:58 + LP])
                    return P.rearrange("p (h w) -> p h w", h=56, w=W)

                stage = {}   # it -> (xv, [P views]) ready for the PE pass
                xts = {}     # it -> loaded x tile awaiting P build
                for it in range(NIT + 1):
                    if it < NIT:
                        cb, dq = divmod(it, DQ)
                        xt = xq.tile([128, 4, HW], BF16, name="xt")
                        if it == 0:
                            # split the first load per d-slice to cut the ramp
                            for dd in range(4):
                                nc.sync.dma_start(
                                    xt[:, dd, :],
                                    x_d[0:128, dd, :])
                        else:
                            nc.sync.dma_start(
                                xt[:], x_d[cb * 128:(cb + 1) * 128,
                                           dq * 4:(dq + 1) * 4, :]
                            )
                        xts[it] = xt
                    if it == 0:
                        xt0 = xts.pop(0)
                        stage[0] = (xt0.rearrange("p dd (h w) -> p dd h w",
                                                  h=H, w=W),
                                    [build_P_dd(0, xt0, dd) for dd in range(4)])
                        continue
                    cb, dq = divmod(it - 1, DQ)
                    xv, Pvs = stage.pop(it - 1)
                    for hb in range(HB):
                        ps = cps.tile([128, 4, 512], F32, tag="ps", name="ps")
                        for dd in range(4):
                            psv = ps[:, dd, 0:448].rearrange(
                                "p (h w) -> p h w", h=8, w=56)
                            for ti, t in enumerate(PE_TAPS):
                                dh, dw = t // 3, t % 3
                                if t == 3:
                                    rhs = Pvs[dd][:, 8 * hb: 8 * hb + 8, 0:56]
                                else:
                                    rhs = xv[:, dd,
                                             8 * hb + dh: 8 * hb + dh + 8,
                                             dw: dw + 56]
                                nc.tensor.matmul(
                                    psv,
                                    dg_sb[:, (cb * 9 + t) * 128:
                                          (cb * 9 + t + 1) * 128],
                                    rhs,
                                    start=(ti == 0), stop=(ti == len(PE_TAPS) - 1),
                                )
                        # fused W/H/D maxpool: (p, wb, dd, h, w) -> (p, wb)
                        rin = ps[:, :, 0:448].rearrange(
                            "p dd (h wb w) -> p wb dd h w", h=8, wb=7, w=8)
                        nc.vector.reduce_max(
                            y_t[cb][:, dq * 49 + hb * 7: dq * 49 + hb * 7 + 7],
                            rin, axis=AX.XYZ)
                        # next quad's carrier builds spread mid-iteration so
                        # the tail reduces aren't queued behind them on DVE
                        if 1 <= hb <= 4 and it in xts:
                            nxt = xts[it]
                            if hb == 1:
                                stage[it] = (nxt.rearrange(
                                    "p dd (h w) -> p dd h w", h=H, w=W), [])
                            stage[it][1].append(build_P_dd(it, nxt, hb - 1))
                            if hb == 4:
                                xts.pop(it)
                    if dq == DQ - 1:
                        # bias + relu on pooled values
                        nc.scalar.activation(y2_t[cb][:], y_t[cb][:], AF.Relu,
                                             bias=bias_sb[:, cb:cb + 1], scale=1.0)

            # ---------------- attention tail (f32) ----------------
            with (
                tc.tile_pool(name="asb", bufs=1) as asb,
                tc.tile_pool(name="aps", bufs=4, space="PSUM") as aps,
            ):
                # grouped 1x1 convs g/f/h: out[g,n] = w0[g] y[2g,n] + w1[g] y[2g+1,n]
                gfh_sb = []   # [w][half] -> (128, 196) f32 sbuf
                for wi in range(3):
                    halves = []
                    for half in range(2):
                        pst = aps.tile([128, N], F32, tag="aps", name=f"gfh{wi}{half}")
                        for sub in range(2):
                            cb = half * 2 + sub
                            nc.tensor.matmul(
                                pst[sub * 64:(sub + 1) * 64, :],
                                attw_sb[:, (wi * 4 + cb) * 64:(wi * 4 + cb + 1) * 64],
                                y2_t[cb][:],
                                start=True, stop=True,
                            )
                        sb = asb.tile([128, N], F32, tag=f"gfhs{wi}{half}", name=f"gfhs{wi}{half}")
                        nc.scalar.copy(sb[:], pst[:])
                        halves.append(sb)
                    gfh_sb.append(halves)
                g_sb, f_sb, h_sb = gfh_sb

                # g reshape (C2,N)->(N,C2) via DRAM round trip (row-major reinterpret)
                gv = gflat_d.rearrange("(c n) -> c n", n=N)
                nc.sync.dma_start(gv[0:128, :], g_sb[0][:])
                nc.sync.dma_start(gv[128:256, :], g_sb[1][:])
                giv = gflat_d.rearrange("(i k) -> i k", k=C2)
                ga = asb.tile([128, C2], F32)   # G rows 0:128
                gb = asb.tile([128, C2], F32)   # G rows 128:196 in [0:68]
                nc.sync.dma_start(ga[:], giv[0:128, :])
                nc.sync.dma_start(gb[0:68, :], giv[128:N, :])

                # G^T via PE transposes: gt[half] = G^T[half*128:...,:196]
                gt_sb = []
                for half in range(2):
                    pst = aps.tile([128, N], F32, tag="aps", name=f"gt{half}")
                    nc.tensor.transpose(
                        pst[:, 0:128], ga[:, half * 128:(half + 1) * 128], ident_sb[:])
                    nc.tensor.transpose(
                        pst[:, 128:N], gb[0:68, half * 128:(half + 1) * 128],
                        ident_sb[0:68, 0:68])
                    sb = asb.tile([128, N], F32, tag=f"gts{half}", name=f"gts{half}")
                    nc.scalar.copy(sb[:], pst[:])
                    gt_sb.append(sb)

                # scores[i,m] = sum_k G^T[k,i] F[k,m]; split i into [0:128),[128:196)
                soft_sb = []
                for mi, (lo, sz) in enumerate(((0, 128), (128, 68))):
                    pst = aps.tile([128, N], F32, tag="aps", name=f"sc{mi}")
                    nc.tensor.matmul(pst[0:sz, :], gt_sb[0][:, lo:lo + sz],
                                     f_sb[0][:], start=True, stop=False)
                    nc.tensor.matmul(pst[0:sz, :], gt_sb[1][:, lo:lo + sz],
                                     f_sb[1][:], start=False, stop=True)
                    # softmax along free dim
                    nmax = asb.tile([128, 1], F32, tag=f"nmax{mi}", name=f"nmax{mi}")
                    nc.vector.reduce_max(nmax[0:sz, :], pst[0:sz, :], axis=AX.X,
                                         negate=True)
                    e = asb.tile([128, N], F32, tag=f"e{mi}", name=f"e{mi}")
                    nc.scalar.activation(e[0:sz, :], pst[0:sz, :], AF.Exp,
                                         bias=nmax[0:sz, :], scale=1.0)
                    ssum = asb.tile([128, 1], F32, tag=f"ssum{mi}", name=f"ssum{mi}")
                    nc.vector.tensor_reduce(ssum[0:sz, :], e[0:sz, :], axis=AX.X,
                                            op=mybir.AluOpType.add)
                    sinv = asb.tile([128, 1], F32, tag=f"sinv{mi}", name=f"sinv{mi}")
                    nc.vector.reciprocal(sinv[0:sz, :], ssum[0:sz, :])
                    nc.vector.tensor_scalar_mul(e[0:sz, :], e[0:sz, :], sinv[0:sz, :])
                    soft_sb.append(e)

                # h^T via PE transposes: ht_a = h^T[n 0:128, c], ht_b = h^T[n 128:196, c]
                ht_a_ps = aps.tile([128, C2], F32, tag="aps")
                nc.tensor.transpose(ht_a_ps[:, 0:128], h_sb[0][:, 0:128], ident_sb[:])
                nc.tensor.transpose(ht_a_ps[:, 128:C2], h_sb[1][:, 0:128], ident_sb[:])
                ht_b_ps = aps.tile([128, C2], F32, tag="aps")
                nc.tensor.transpose(ht_b_ps[0:68, 0:128], h_sb[0][:, 128:N],
                                    ident_sb[:])
                nc.tensor.transpose(ht_b_ps[0:68, 128:C2], h_sb[1][:, 128:N],
                                    ident_sb[:])
                ht_a = asb.tile([128, C2], F32)
                ht_b = asb.tile([128, C2], F32)
                nc.scalar.copy(ht_a[:], ht_a_ps[:])
                nc.scalar.copy(ht_b[0:68, :], ht_b_ps[0:68, :])

                # out[c,m] = sum_n h^T[n,c] soft[n,m]
                for mi, (lo, sz) in enumerate(((0, 128), (128, 128))):
                    pst = aps.tile([128, N], F32, tag="aps", name=f"o{mi}")
                    nc.tensor.matmul(pst[:], ht_a[:, lo:lo + sz], soft_sb[0][:],
                                     start=True, stop=False)
                    nc.tensor.matmul(pst[:], ht_b[0:68, lo:lo + sz],
                                     soft_sb[1][0:68, :], start=False, stop=True)
                    osb = asb.tile([128, N], F32, tag=f"os{mi}", name=f"os{mi}")
                    nc.scalar.copy(osb[:], pst[:])
                    nc.sync.dma_start(out_d[lo:lo + sz, :], osb[:])

    nc.compile()
    return nc


def _host_prep(conv1_w, conv1_b, gamma, beta, r_mean, r_var, wg, wf, wh):
    inv = gamma / np.sqrt(r_var + EPS)                       # (C,)
    w9 = conv1_w.reshape(C, 9) * inv[:, None]                # BN scale folded
    bias = (conv1_b - r_mean) * inv + beta                   # (C,)

    # carrier tap 3=(1,0): clamp |w3| away from 0 so fold ratios stay finite
    w3 = w9[:, 3].copy()
    tiny = np.abs(w3) < 1e-6
    w3[tiny] = np.where(w3[tiny] >= 0, 1e-6, -1e-6)
    w9 = w9.copy()
    w9[:, 3] = w3
    r0 = w9[:, 0] / w3
    r6 = w9[:, 6] / w3
    r2 = w9[:, 2] / w3

    dg = np.zeros((128, CB * 9 * 128), np.float32)
    for cb in range(CB):
        for t in range(9):
            col = (cb * 9 + t) * 128
            dg[np.arange(128), col + np.arange(128)] = w9[cb * 128:(cb + 1) * 128, t]
    dg = dg.astype(ml_dtypes.bfloat16)

    rvec = np.zeros((128, 12), np.float32)
    for cb in range(CB):
        rvec[:, 2 * cb] = r0[cb * 128:(cb + 1) * 128]
        rvec[:, 2 * cb + 1] = r6[cb * 128:(cb + 1) * 128]
        rvec[:, 8 + cb] = r2[cb * 128:(cb + 1) * 128]

    bias_a = bias.reshape(CB, 128).T.astype(np.float32).copy()  # (128, CB)

    attw = np.zeros((128, 12 * 64), np.float32)
    for wi, wmat in enumerate((wg, wf, wh)):
        for cb in range(CB):
            col = (wi * 4 + cb) * 64
            j = np.arange(64)
            attw[2 * j, col + j] = wmat[64 * cb + j, 0]
            attw[2 * j + 1, col + j] = wmat[64 * cb + j, 1]

    ident = np.eye(128, dtype=np.float32)
    return dg, bias_a, attw, ident, rvec


def kernel(**inputs):
    x = np.ascontiguousarray(
        np.asarray(inputs["x"], dtype=np.float32)).astype(ml_dtypes.bfloat16)
    args = [np.asarray(inputs[k], dtype=np.float32) for k in
            ("conv1_w", "conv1_b", "gamma", "beta", "r_mean", "r_var",
             "wg", "wf", "wh")]
    dg, bias_a, attw, ident, rvec = _host_prep(*args)

    if "nc" not in _CACHE:
        _CACHE["nc"] = _build_nc()
    nc = _CACHE["nc"]

    in_maps = [
        {"x": x[b].reshape(C, D, HW), "dg": dg, "bias": bias_a,
         "attw": attw, "ident": ident, "rvec": rvec}
        for b in range(B)
    ]
    res = run_bass_kernel_spmd(nc, in_maps, list(range(B)),
                               **_CACHE.get("run_kwargs", {}))
    _CACHE["last_results"] = res
    out = np.stack([res.results[b]["out"].reshape(C2, Do, Ho, Wo)
                    for b in range(B)])
    return out.astype(np.float32)

